# revision 11
# baseline (speedup 1.0000x reference)
"""ClippedGRU Trainium2 kernel.

Strategy (per spec sharding hint): data-parallel over batch across 8 cores
(B_local=32). Each core runs the sequential T-step scan locally, split into
2 interleaved batch-16 streams so one stream's matmuls hide the other
stream's serial elementwise chain.

Per stream-step dataflow (all on one core):
  - PE: 4 transposes of previous h' (batch-layout -> hT chunks), then
    xi matmuls (x_t @ W_ih^T, K=128) and recurrent matmuls
    (h @ W_hh^T, K=512 as 4 psum-accumulated chunks), all fp32r
    (full-rate, ~2e-4 rel err) with N=512 tiles.
  - ACT: sigmoid(r), sigmoid(z), sigmoid(-u)=1-z, tanh — one table set.
  - DVE: q=(hn+bhh_n)*r, s=(xn+bih_n)+q, f=(1-z)*n, pre=f+z*h,
    h'=clip(pre) fused min/max, psum->sbuf transpose copy.
  - GPSIMD: e1 = z*h.
Biases: when each gate-bias slice is constant (true for this module:
b_ih=[1]*2H+[0]*H, b_hh=0) they fold into ACT/DVE scalar immediates; else a
K=1 ones-row matmul per psum bank adds the bias vector.
"""
import sys

if "/opt/trn_rl_repo" not in sys.path:
    sys.path.insert(0, "/opt/trn_rl_repo")

import numpy as np

B, T_FULL, I, H = 256, 500, 128, 512
CLIP = 5.0
NCORES = 8
BL = B // NCORES   # 32 rows per core
NS = 2             # interleaved streams per core
BS = BL // NS      # 16 rows per stream
KC = H // 128      # 4 contraction chunks for W_hh
G3 = 3 * H         # 1536

_CACHE = {}


def _build(T, imm_bias, bias_c):
    import concourse.bass as bass
    import concourse.tile as tile
    from concourse import bacc, mybir

    F32 = mybir.dt.float32
    F32R = mybir.dt.float32r
    AF = mybir.ActivationFunctionType
    OP = mybir.AluOpType

    nc = bacc.Bacc("TRN2", target_bir_lowering=False, debug=False,
                   num_devices=NCORES)

    # ACT bias immediates are lowered via the const-AP pool; register ours.
    cr, cz, chn, cxn = bias_c
    for val in {cr, cz, -cz, 0.0}:
        if (F32, val) not in nc.const_aps.aps:
            ct = nc.alloc_sbuf_tensor(f"const-float32-{val}", [128, 1], F32)
            nc.gpsimd.memset(ct.ap(), val)
            nc.const_aps.aps[(F32, val)] = ct.ap()
    nc.all_engine_barrier()

    xT_d = nc.dram_tensor("xT", [128, T, BL], F32R, kind="ExternalInput").ap()
    h0T_d = nc.dram_tensor("h0T", [128, KC, BL], F32R, kind="ExternalInput").ap()
    h0b_d = nc.dram_tensor("h0b", [NS, BS, H], F32, kind="ExternalInput").ap()
    whhT_d = nc.dram_tensor("whhT", [128, KC, G3], F32R, kind="ExternalInput").ap()
    wihT_d = nc.dram_tensor("wihT", [128, G3], F32R, kind="ExternalInput").ap()
    ident_d = nc.dram_tensor("ident", [BS, BS], F32, kind="ExternalInput").ap()
    if not imm_bias:
        brow_d = nc.dram_tensor("brow", [1, 4 * H], F32R, kind="ExternalInput").ap()
        ones_d = nc.dram_tensor("ones", [1, BL], F32R, kind="ExternalInput").ap()
    out_d = nc.dram_tensor("out", [BL, T, H], F32, kind="ExternalOutput").ap()

    with tile.TileContext(nc) as tc:
        with tc.tile_pool(name="const", bufs=1) as cpool, \
             tc.tile_pool(name="work", bufs=2) as wk, \
             tc.tile_pool(name="state", bufs=2 * NS) as st, \
             tc.tile_pool(name="psum", bufs=2, space="PSUM") as ps:

            xT = cpool.tile([128, T, BL], F32R)
            h0T = cpool.tile([128, KC, BL], F32R)
            whhT = cpool.tile([128, KC, G3], F32R)
            wihT = cpool.tile([128, G3], F32R)
            ident = cpool.tile([BS, BS], F32)
            nc.sync.dma_start(whhT[:], whhT_d)
            nc.sync.dma_start(wihT[:], wihT_d)
            nc.sync.dma_start(h0T[:], h0T_d)
            nc.sync.dma_start(ident[:], ident_d)
            nc.sync.dma_start(xT[:], xT_d)
            if not imm_bias:
                brow = cpool.tile([1, 4 * H], F32R)
                ones = cpool.tile([1, BL], F32R)
                nc.sync.dma_start(brow[:], brow_d)
                nc.sync.dma_start(ones[:], ones_d)

            # per-stream rolling state handles
            h_prev = [None] * NS    # [BS, H] fp32, batch layout
            for s in range(NS):
                h_init = st.tile([BS, H], F32, tag="h")
                nc.sync.dma_start(h_init[:], h0b_d[s])
                h_prev[s] = h_init

            for t in range(T):
                for s in range(NS):
                    c0, c1 = s * BS, (s + 1) * BS
                    xsl = xT[:, t, c0:c1]

                    ps_r = ps.tile([BS, H], F32, tag="ps_r")
                    ps_z = ps.tile([BS, H], F32, tag="ps_z")
                    ps_xn = ps.tile([BS, H], F32, tag="ps_xn")
                    ps_hn = ps.tile([BS, H], F32, tag="ps_hn")

                    # --- input-projection matmuls (independent of h) ---
                    if imm_bias:
                        nc.tensor.matmul(ps_r[:], xsl, wihT[:, 0:H],
                                         start=True, stop=False)
                        nc.tensor.matmul(ps_z[:], xsl, wihT[:, H:2 * H],
                                         start=True, stop=False)
                        nc.tensor.matmul(ps_xn[:], xsl, wihT[:, 2 * H:G3],
                                         start=True, stop=True)
                    else:
                        on = ones[:, 0:BS]
                        nc.tensor.matmul(ps_r[:], on, brow[:, 0:H],
                                         start=True, stop=False)
                        nc.tensor.matmul(ps_z[:], on, brow[:, H:2 * H],
                                         start=True, stop=False)
                        nc.tensor.matmul(ps_xn[:], on, brow[:, 3 * H:4 * H],
                                         start=True, stop=False)
                        nc.tensor.matmul(ps_hn[:], on, brow[:, 2 * H:3 * H],
                                         start=True, stop=False)
                        nc.tensor.matmul(ps_r[:], xsl, wihT[:, 0:H],
                                         start=False, stop=False)
                        nc.tensor.matmul(ps_z[:], xsl, wihT[:, H:2 * H],
                                         start=False, stop=False)
                        nc.tensor.matmul(ps_xn[:], xsl, wihT[:, 2 * H:G3],
                                         start=False, stop=True)

                    # --- transpose prev h' into hT chunks (PE) ---
                    if t == 0:
                        hT = h0T[:, :, c0:c1]
                    else:
                        trp = ps.tile([128, KC * BS], F32, tag="ps_xn")
                        for c in range(KC):
                            nc.tensor.transpose(
                                trp[:, c * BS:(c + 1) * BS],
                                h_prev[s][:, c * 128:(c + 1) * 128],
                                ident[:])
                        hTs = st.tile([128, KC, BS], F32R, tag="hT")
                        nc.vector.tensor_copy(
                            hTs[:].rearrange("p a b -> p (a b)"), trp[:])
                        hT = hTs[:]

                    # --- recurrent matmuls ---
                    for c in range(KC):
                        nc.tensor.matmul(ps_r[:], hT[:, c, :],
                                         whhT[:, c, 0:H],
                                         start=False, stop=(c == KC - 1))
                    for c in range(KC):
                        nc.tensor.matmul(ps_hn[:], hT[:, c, :],
                                         whhT[:, c, 2 * H:G3],
                                         start=(imm_bias and c == 0),
                                         stop=(c == KC - 1))
                    for c in range(KC):
                        nc.tensor.matmul(ps_z[:], hT[:, c, :],
                                         whhT[:, c, H:2 * H],
                                         start=False, stop=(c == KC - 1))

                    hb = h_prev[s][:]

                    # --- elementwise chain ---
                    r = wk.tile([BS, H], F32, tag="r")
                    nc.scalar.activation(r[:], ps_r[:], AF.Sigmoid, bias=cr)
                    q = wk.tile([BS, H], F32, tag="q")
                    nc.vector.scalar_tensor_tensor(
                        q[:], ps_hn[:], chn, r[:], op0=OP.add, op1=OP.mult)
                    sarg = wk.tile([BS, H], F32, tag="sarg")
                    nc.vector.scalar_tensor_tensor(
                        sarg[:], ps_xn[:], cxn, q[:], op0=OP.add, op1=OP.add)
                    n = wk.tile([BS, H], F32, tag="n")
                    nc.scalar.activation(n[:], sarg[:], AF.Tanh)

                    z = wk.tile([BS, H], F32, tag="z")
                    nc.scalar.activation(z[:], ps_z[:], AF.Sigmoid, bias=cz)
                    w1z = wk.tile([BS, H], F32, tag="w1z")
                    nc.scalar.activation(w1z[:], ps_z[:], AF.Sigmoid,
                                         bias=-cz, scale=-1.0)
                    e1 = wk.tile([BS, H], F32, tag="e1")
                    nc.gpsimd.tensor_tensor(e1[:], z[:], hb, op=OP.mult)

                    f = wk.tile([BS, H], F32, tag="f")
                    nc.vector.tensor_tensor(f[:], w1z[:], n[:], op=OP.mult)
                    pre = wk.tile([BS, H], F32, tag="pre")
                    nc.vector.tensor_tensor(pre[:], f[:], e1[:], op=OP.add)
                    hnew = st.tile([BS, H], F32, tag="h")
                    nc.vector.tensor_scalar(hnew[:], pre[:], CLIP, -CLIP,
                                            op0=OP.min, op1=OP.max)

                    nc.sync.dma_start(out_d[c0:c1, t, :], hnew[:])
                    h_prev[s] = hnew

    nc.compile()
    return nc


def _prep_core(x_c, h0_c, whhT_p, wihT_p, brow, imm_bias, T):
    m = {
        "xT": np.ascontiguousarray(x_c.transpose(2, 1, 0)),          # [I,T,BL]
        "h0T": np.ascontiguousarray(
            h0_c.T.reshape(KC, 128, BL).transpose(1, 0, 2)),          # [128,KC,BL]
        "h0b": np.ascontiguousarray(h0_c.reshape(NS, BS, H)),
        "whhT": whhT_p,
        "wihT": wihT_p,
        "ident": np.eye(BS, dtype=np.float32),
    }
    if not imm_bias:
        m["brow"] = brow
        m["ones"] = np.ones((1, BL), np.float32)
    return m


def kernel(x, h_0, w_ih, w_hh, b_ih, b_hh, T=None):
    from concourse.bass_utils import run_bass_kernel_spmd

    T = x.shape[1] if T is None else T
    x = np.asarray(x, np.float32)[:, :T]
    h_0 = np.asarray(h_0, np.float32)
    w_ih = np.asarray(w_ih, np.float32)
    w_hh = np.asarray(w_hh, np.float32)
    b_ih = np.asarray(b_ih, np.float32)
    b_hh = np.asarray(b_hh, np.float32)

    bsum = b_ih + b_hh
    slices = [bsum[0:H], bsum[H:2 * H], b_hh[2 * H:], b_ih[2 * H:]]
    imm_bias = all(np.ptp(sl) == 0.0 for sl in slices)
    bias_c = tuple(float(sl[0]) if imm_bias else 0.0 for sl in slices)

    key = (T, imm_bias, bias_c)
    if key not in _CACHE:
        _CACHE[key] = _build(T, imm_bias, bias_c)
    nc = _CACHE[key]

    whhT_p = np.ascontiguousarray(
        w_hh.T.reshape(KC, 128, G3).transpose(1, 0, 2))
    wihT_p = np.ascontiguousarray(w_ih.T)
    brow = None
    if not imm_bias:
        brow = np.concatenate(
            [bsum[0:2 * H], b_hh[2 * H:], b_ih[2 * H:]]).reshape(1, 4 * H)
        brow = np.ascontiguousarray(brow, np.float32)

    in_maps = []
    for c in range(NCORES):
        rows = slice(c * BL, (c + 1) * BL)
        in_maps.append(_prep_core(x[rows], h_0[rows], whhT_p, wihT_p,
                                  brow, imm_bias, T))

    global _last_in_maps
    _last_in_maps = in_maps
    res = run_bass_kernel_spmd(nc, in_maps, core_ids=list(range(NCORES)))
    out = np.concatenate([res.results[c]["out"] for c in range(NCORES)], axis=0)
    h_last = np.ascontiguousarray(out[:, -1, :])
    return out, h_last
